# revision 25
# baseline (speedup 1.0000x reference)
"""Single-head causal attention (B=128, T=512, C=256, H=64) on 8 trn2 cores.

Data-parallel: 16 batches per core. Per batch, on-chip dataflow:
  x [512,256] --SWDGE cast--> x_bf16 --PE transpose--> xT [256,512]
  qT/kT = W^T @ xT   (bf16 matmul, fp32 psum; 1/sqrt(H) folded into Wq)
  simT[s,t] = kT.T-slice @ qT   (only t >= 128*floor(s/128) computed)
  pT = exp(simT)  (ACT, psum->sbuf, bf16 out), causal diag masked via 0/1 mult
  v = x @ Wv  (natural [s,h] layout, ones column appended)
  out_unnorm[t,h], rowsum[t] = pT.T @ [v|1]  (AV, fp32 psum)
  out = out_unnorm * recip(rowsum)  -> DMA

Batches are processed in pairs: batch j of a pair keeps qT/kT on SBUF
partitions 64j..64j+63 so the two sims use distinct PE row-groups.
"""
import numpy as np
import ml_dtypes

B, T, C, H = 128, 512, 256, 64
N_CORES = 8
BL = B // N_CORES          # batches per core
TC = T // 128              # 4 t-chunks
CS = C // 128              # 2 c-subtiles
INV_SQRT_H = 1.0 / np.sqrt(H)


def _build_program():
    import concourse.tile as tile
    from concourse import bacc, mybir

    dt = mybir.dt
    nc = bacc.Bacc("TRN2", target_bir_lowering=False, debug=False,
                   enable_asserts=False, num_devices=N_CORES)

    x_d = nc.dram_tensor("x", [BL, T, C], dt.float32, kind="ExternalInput").ap()
    wq_d = nc.dram_tensor("wq8", [CS, 128, H], dt.bfloat16, kind="ExternalInput").ap()
    wk_d = nc.dram_tensor("wk", [CS, 128, H], dt.bfloat16, kind="ExternalInput").ap()
    wv_d = nc.dram_tensor("wv", [CS, 128, H], dt.bfloat16, kind="ExternalInput").ap()
    id_d = nc.dram_tensor("ident", [128, 128], dt.bfloat16, kind="ExternalInput").ap()
    out_d = nc.dram_tensor("out", [BL, T, H], dt.float32, kind="ExternalOutput").ap()

    with tile.TileContext(nc) as tc:
        from contextlib import ExitStack
        ctx = ExitStack()
        with ctx:
            consts = ctx.enter_context(tc.tile_pool(name="consts", bufs=1))
            sb_x = ctx.enter_context(tc.tile_pool(name="sb_x", bufs=4))
            sb_xt = ctx.enter_context(tc.tile_pool(name="sb_xt", bufs=4))
            sb_qk = ctx.enter_context(tc.tile_pool(name="sb_qk", bufs=4))
            sb_p = ctx.enter_context(tc.tile_pool(name="sb_p", bufs=4))
            sb_v = ctx.enter_context(tc.tile_pool(name="sb_v", bufs=4))
            sb_o = ctx.enter_context(tc.tile_pool(name="sb_o", bufs=6))
            ps_xt = ctx.enter_context(tc.tile_pool(name="ps_xt", bufs=2, space="PSUM"))
            ps_qk = ctx.enter_context(tc.tile_pool(name="ps_qk", bufs=1, space="PSUM"))
            ps_sim = ctx.enter_context(tc.tile_pool(name="ps_sim", bufs=2, space="PSUM"))
            ps_v = ctx.enter_context(tc.tile_pool(name="ps_v", bufs=1, space="PSUM"))
            ps_av = ctx.enter_context(tc.tile_pool(name="ps_av", bufs=1, space="PSUM"))

            wq_sb = consts.tile([128, CS, H], dt.bfloat16)
            nc.sync.dma_start(wq_sb[:], wq_d.rearrange("cs p h -> p cs h"))
            wk_sb = consts.tile([128, CS, H], dt.bfloat16)
            nc.sync.dma_start(wk_sb[:], wk_d.rearrange("cs p h -> p cs h"))
            wv_sb = consts.tile([128, CS, H], dt.bfloat16)
            nc.sync.dma_start(wv_sb[:], wv_d.rearrange("cs p h -> p cs h"))
            id_sb = consts.tile([128, 128], dt.bfloat16)
            nc.sync.dma_start(id_sb[:], id_d)
            # maskneg[s,t] = -1e30 where t < s (strictly below diag), else 0.
            # Added onto sim diag blocks in PSUM via I.T @ maskneg so exp -> 0.
            mn_d = nc.dram_tensor("maskneg", [128, 128], dt.bfloat16,
                                  kind="ExternalInput").ap()
            mn_sb = consts.tile([128, 128], dt.bfloat16)
            nc.sync.dma_start(mn_sb[:], mn_d)

            for pair in range(BL // 2):
                # one SWDGE cast-DMA loads both batches of the pair
                xbf2 = sb_x.tile([128, 2, TC, C], dt.bfloat16,
                                 name=f"xbf{pair}", tag="xbf")
                nc.gpsimd.dma_start(
                    xbf2[:], x_d[2 * pair:2 * pair + 2].rearrange(
                        "b (tc p) c -> p b tc c", p=128))
                xts = []
                for j in range(2):
                    b = 2 * pair + j
                    xbf = xbf2[:, j]
                    # transpose -> psum bf16 [128(c), 2(cc), 512(t)]
                    pxt = ps_xt.tile([128, CS, T], dt.bfloat16, name=f"pxt{b}",
                                     tag="pxt")
                    for cc in range(CS):
                        for tci in range(TC):
                            nc.tensor.transpose(
                                pxt[:, cc, 128 * tci:128 * (tci + 1)],
                                xbf[:, tci, 128 * cc:128 * (cc + 1)],
                                id_sb[:])
                    xt = sb_xt.tile([128, CS, T], dt.bfloat16, name=f"xt{b}",
                                    tag="xt")
                    nc.vector.tensor_copy(xt[:], pxt[:])
                    xts.append(xt)

                # q/k projections for the pair: batch j on partitions 64j..64j+63
                pqk = ps_qk.tile([128, 2, T], dt.float32, name=f"pqk{pair}",
                                 tag="pqk")
                for j in range(2):
                    for i, w_sb in enumerate((wq_sb, wk_sb)):
                        for cc in range(CS):
                            nc.tensor.matmul(
                                pqk[64 * j:64 * (j + 1), i, :],
                                w_sb[:, cc, :], xts[j][:, cc, :],
                                start=(cc == 0), stop=(cc == CS - 1),
                                tile_position=(0, 64 * j))
                qk = sb_qk.tile([128, 2, T], dt.bfloat16, name=f"qk{pair}",
                                tag="qk")
                nc.vector.tensor_copy(qk[:, 0, :], pqk[:, 0, :])
                nc.scalar.copy(qk[:, 1, :], pqk[:, 1, :])

                for j in range(2):
                    b = 2 * pair + j
                    xt = xts[j]
                    qT = qk[64 * j:64 * (j + 1), 0, :]
                    kT = qk[64 * j:64 * (j + 1), 1, :]

                    # v = x @ Wv, natural [s, h] layout + ones column
                    pv = ps_v.tile([128, TC, H], dt.float32, name=f"pv{b}",
                                   tag="pv")
                    for sc in range(TC):
                        for cc in range(CS):
                            nc.tensor.matmul(
                                pv[:, sc, :],
                                xt[:, cc, 128 * sc:128 * (sc + 1)],
                                wv_sb[:, cc, :],
                                start=(cc == 0), stop=(cc == CS - 1))
                    v1 = sb_v.tile([128, TC, H + 1], dt.bfloat16, name=f"v1{b}",
                                   tag="v1")
                    nc.vector.tensor_copy(v1[:, :, 0:H], pv[:])
                    nc.gpsimd.memset(v1[:, :, H:H + 1], 1.0)

                    # simT + exp -> pT (packed si-major), diag mask
                    pT = sb_p.tile([128, 1280], dt.bfloat16, name=f"pT{b}",
                                   tag="pT")
                    offs = []
                    off = 0
                    for si in range(TC):
                        n_si = T - 128 * si
                        offs.append(off)
                        psim = ps_sim.tile([128, T], dt.float32,
                                           name=f"psim{b}_{si}", tag="psim")
                        nc.tensor.matmul(
                            psim[:, 0:128],
                            id_sb[:], mn_sb[:],
                            start=True, stop=False,
                            skip_group_check=True)
                        nc.tensor.matmul(
                            psim[:, 0:n_si],
                            kT[:, 128 * si:128 * (si + 1)],
                            qT[:, 128 * si:T],
                            start=False, stop=True,
                            tile_position=(64 * j, 0),
                            skip_group_check=True)
                        nc.scalar.activation(
                            pT[:, off:off + n_si], psim[:, 0:n_si],
                            mybir.ActivationFunctionType.Exp,
                            scale=float(INV_SQRT_H))
                        off += n_si

                    # AV: out[t-chunk, 0:64]=sum_s p v ; col 64 = rowsum
                    pav = ps_av.tile([128, TC, H + 1], dt.float32,
                                     name=f"pav{b}", tag="pav")
                    for ci in range(TC):
                        for si in range(ci + 1):
                            nc.tensor.matmul(
                                pav[:, ci, :],
                                pT[:, offs[si] + 128 * (ci - si):
                                   offs[si] + 128 * (ci - si) + 128],
                                v1[:, si, :],
                                start=(si == 0), stop=(si == ci))
                    rec = sb_o.tile([128, TC], dt.float32, name=f"rec{b}",
                                    tag="rec")
                    nc.vector.reciprocal(rec[:], pav[:, :, H])
                    osb = sb_o.tile([128, TC, H], dt.float32, name=f"osb{b}",
                                    tag="osb")
                    nc.vector.tensor_mul(
                        out=osb[:],
                        in0=pav[:, :, 0:H],
                        in1=rec[:, :, None].to_broadcast([128, TC, H]))
                    nc.sync.dma_start(
                        out_d[b].rearrange("(tc p) h -> p tc h", p=128), osb[:])

    nc.compile()
    return nc


_CACHED = None


def _get_program():
    global _CACHED
    if _CACHED is None:
        _CACHED = _build_program()
    return _CACHED


def _host_inputs(Wq, Wk, Wv):
    bf16 = ml_dtypes.bfloat16
    # 1/sqrt(H) is applied as the exp() input scale, not folded into Wq.
    consts = {
        "wq8": np.ascontiguousarray(np.asarray(Wq, np.float32).reshape(CS, 128, H)).astype(bf16),
        "wk": np.ascontiguousarray(np.asarray(Wk, np.float32).reshape(CS, 128, H)).astype(bf16),
        "wv": np.ascontiguousarray(np.asarray(Wv, np.float32).reshape(CS, 128, H)).astype(bf16),
        "ident": np.eye(128, dtype=np.float32).astype(bf16),
        "maskneg": np.where(np.arange(128)[None, :] < np.arange(128)[:, None],
                            np.float32(-1e30), np.float32(0)).astype(bf16),
    }
    return consts


def kernel(input_embeddings, Wq, Wk, Wv):
    from concourse.bass_utils import run_bass_kernel_spmd

    x = np.ascontiguousarray(np.asarray(input_embeddings, np.float32))
    nc = _get_program()
    consts = _host_inputs(Wq, Wk, Wv)
    in_maps = []
    for c in range(N_CORES):
        m = {"x": x[c * BL:(c + 1) * BL]}
        m.update(consts)
        in_maps.append(m)
    res = run_bass_kernel_spmd(nc, in_maps, core_ids=list(range(N_CORES)))
    out = np.concatenate([res.results[c]["out"] for c in range(N_CORES)], axis=0)
    return out.astype(np.float32)


if __name__ == "__main__":
    rng = np.random.default_rng(0)
    x = rng.standard_normal((B, T, C)).astype(np.float32)
    wq = (rng.standard_normal((C, H)) / 16).astype(np.float32)
    wk = (rng.standard_normal((C, H)) / 16).astype(np.float32)
    wv = (rng.standard_normal((C, H)) / 16).astype(np.float32)
    out = kernel(x, wq, wk, wv)
    print("out", out.shape, out.dtype)
